# revision 5
# baseline (speedup 1.0000x reference)
"""Trainium2 Bass kernel for nn_AuxCMP_61907658604772 (retrieval_knn).

Reference semantics (only the last time step of d/m matters):
    data = d[:, -1].reshape(B, C, S2)            # [64, 64, 1024] f32
    mask = m[:, -1].reshape(B, C, S2)            # [64, 64, 1024] i32 (0/1)
    cell_empty = (mask.sum(axis=(0, 1)) == 0)    # [1024] per-cell predicate
    gathered = data[:, :, poi_index]             # gather along cell dim
    out = (data + where(cell_empty, gathered, 0)).reshape(B, C, 32, 32)

Sharding: data-parallel over batch B across 8 cores (8 batches/core).
Device layout is cell-major ("transposed"): per core
    data_t [1024 cells, 512 rows]  f32   (rows = b_local*64 + c)
    mask_t [1024 cells, 512 rows]  i8
so the per-cell mask reduction is a free-dim DVE reduce, the per-cell
predicate multiply is a per-partition broadcast, and the poi gather is a row
gather done with GPSIMD indirect DMA from DRAM.  The per-cell partial sums
are combined with an AllReduce over the 8 cores.

SBUF cell tile layout: tile t holds cells [t*128, (t+1)*128) with cell
t*128+p on partition p; the [128, NT] companion tensors (idx, msums, empty)
hold cell t*128+p at [p, t].
"""

import numpy as np

from concourse import bacc, bass, mybir, tile
from concourse.bass_utils import run_bass_kernel_spmd

N_CORES = 8
B, T, C, S2 = 64, 12, 64, 1024
SIDE = 32
B_LOC = B // N_CORES            # 8 batches per core
ROWS = B_LOC * C                # 512 rows per core
P = 128                         # SBUF partitions
NT = S2 // P                    # 8 cell tiles of 128 cells

_CACHE = {}


def _build_program():
    nc = bacc.Bacc(
        "TRN2",
        target_bir_lowering=False,
        debug=False,
        num_devices=N_CORES,
    )
    data_t = nc.dram_tensor(
        "data_t", [S2, ROWS], mybir.dt.float32, kind="ExternalInput"
    ).ap()
    mask_t = nc.dram_tensor(
        "mask_t", [S2, ROWS], mybir.dt.int8, kind="ExternalInput"
    ).ap()
    idx = nc.dram_tensor("idx", [P, NT], mybir.dt.int32, kind="ExternalInput").ap()
    out_t = nc.dram_tensor(
        "out_t", [S2, ROWS], mybir.dt.float32, kind="ExternalOutput"
    ).ap()

    # DRAM [1024, 512] viewed as [128 partitions, 8 tiles, 512 rows]
    mask_3d = mask_t.rearrange("(t p) e -> p t e", p=P)
    data_3d = data_t.rearrange("(t p) e -> p t e", p=P)
    out_3d = out_t.rearrange("(t p) e -> p t e", p=P)

    with tile.TileContext(nc) as tc:
        with (
            tc.tile_pool(name="sbuf", bufs=1) as pool,
            tc.tile_pool(name="dram", bufs=1, space="DRAM") as dram,
        ):
            idx_sb = pool.tile([P, NT], mybir.dt.int32, tag="idx")
            nc.sync.dma_start(out=idx_sb[:], in_=idx[:])

            # ---- per-cell partial mask sums (one DMA + one reduce) ----
            m_all = pool.tile([P, NT, ROWS], mybir.dt.int8, tag="mask")
            nc.sync.dma_start(out=m_all[:], in_=mask_3d)
            msums = pool.tile([P, NT], mybir.dt.float32, tag="msums")
            nc.vector.tensor_reduce(
                out=msums[:],
                in_=m_all[:],
                axis=mybir.AxisListType.X,
                op=mybir.AluOpType.add,
            )

            # ---- AllReduce the [128, 8] partial sums across the 8 cores ----
            bounce_in = dram.tile([P, NT], mybir.dt.float32)
            bounce_out = dram.tile([P, NT], mybir.dt.float32)
            nc.gpsimd.dma_start(out=bounce_in[:], in_=msums[:])
            nc.gpsimd.collective_compute(
                "AllReduce",
                mybir.AluOpType.add,
                replica_groups=[list(range(N_CORES))],
                ins=[bounce_in[:].opt()],
                outs=[bounce_out[:].opt()],
            )
            summ = pool.tile([P, NT], mybir.dt.float32, tag="summ")
            nc.sync.dma_start(out=summ[:], in_=bounce_out[:])

            # empty[p, t] = 1.0 where the global sum for cell t*128+p is 0
            empty = pool.tile([P, NT], mybir.dt.float32, tag="empty")
            nc.vector.tensor_scalar(
                out=empty[:],
                in0=summ[:],
                scalar1=0.0,
                scalar2=None,
                op0=mybir.AluOpType.is_equal,
            )

            # ---- data load + gather (overlap with the collective) ----
            dts = []
            gts = []
            for t in range(NT):
                dt_ = pool.tile([P, ROWS], mybir.dt.float32, tag=f"d{t}")
                nc.sync.dma_start(out=dt_[:], in_=data_3d[:, t, :])
                gt = pool.tile([P, ROWS], mybir.dt.float32, tag=f"g{t}")
                nc.gpsimd.indirect_dma_start(
                    out=gt[:],
                    out_offset=None,
                    in_=data_t[:, :],
                    in_offset=bass.IndirectOffsetOnAxis(
                        ap=idx_sb[:, t : t + 1], axis=0
                    ),
                )
                dts.append(dt_)
                gts.append(gt)

            # ---- out = data + empty * gathered; mul on ACT, add on DVE ----
            for t in range(NT):
                gt, dt_ = gts[t], dts[t]
                nc.scalar.mul(gt[:], gt[:], empty[:, t : t + 1])
                nc.vector.tensor_add(out=gt[:], in0=dt_[:], in1=gt[:])
                nc.sync.dma_start(out=out_3d[:, t, :], in_=gt[:])

    nc.compile()
    return nc


def _get_program():
    if "nc" not in _CACHE:
        _CACHE["nc"] = _build_program()
    return _CACHE["nc"]


def _marshal(d, m, poi_index):
    d = np.asarray(d)
    m = np.asarray(m)
    poi_index = np.asarray(poi_index)

    dlast = d[:, -1].reshape(B, C, S2)  # [64, 64, 1024] f32
    mlast = m[:, -1].reshape(B, C, S2)  # [64, 64, 1024] i32

    idx_arr = np.ascontiguousarray(
        poi_index.astype(np.int32).reshape(NT, P).T
    )  # [128, 8]; idx_arr[p, t] = poi[t*128 + p]

    in_maps = []
    for k in range(N_CORES):
        data_k = np.ascontiguousarray(
            dlast[k * B_LOC : (k + 1) * B_LOC].reshape(ROWS, S2).T
        ).astype(np.float32)  # [1024, 512]
        mask_k = np.ascontiguousarray(
            mlast[k * B_LOC : (k + 1) * B_LOC].reshape(ROWS, S2).T
        ).astype(np.int8)  # [1024, 512]
        in_maps.append({"data_t": data_k, "mask_t": mask_k, "idx": idx_arr})
    return in_maps


def _unmarshal(results):
    # results[k]["out_t"] is [1024 cells, 512 rows]; rows = b_local*64 + c.
    parts = [
        np.asarray(r["out_t"]).T.reshape(B_LOC, C, S2) for r in results
    ]
    out = np.concatenate(parts, axis=0)  # [64, 64, 1024]
    return np.ascontiguousarray(out.reshape(B, C, SIDE, SIDE).astype(np.float32))


def run(d, m, poi_index, side, trace=False):
    """Run the Bass kernel; returns (output, BassKernelResults)."""
    nc = _get_program()
    in_maps = _marshal(d, m, poi_index)
    res = run_bass_kernel_spmd(
        nc, in_maps, list(range(N_CORES)), trace=trace
    )
    return _unmarshal(res.results), res


def kernel(d, m, poi_index, side):
    out, _ = run(d, m, poi_index, side)
    return out


# revision 20
# speedup vs baseline: 3.0566x; 3.0566x over previous
"""Trainium2 Bass kernel for nn_AuxCMP_61907658604772 (retrieval_knn).

Reference semantics (only the last time step of d/m matters):
    data = d[:, -1].reshape(B, C, S2)            # [64, 64, 1024] f32
    mask = m[:, -1].reshape(B, C, S2)            # [64, 64, 1024] i32 (0/1)
    cell_empty = (mask.sum(axis=(0, 1)) == 0)    # [1024] per-cell predicate
    gathered = data[:, :, poi_index]             # gather along cell dim
    out = (data + where(cell_empty, gathered, 0)).reshape(B, C, 32, 32)

Sharding: by CELLS — core k owns cells [128k, 128(k+1)) x all 4096 (b, c)
rows, in cell-major ("transposed") layout:
    data_half  [2048, 2048] f32  transposed d[:, -1], half-row view (replicated)
    data_slice [128, 4096]  f32  the core's own cell rows (shard)
    maskp      [128, 512]   u8   bit-packed mask rows for the core's cells
    idx2       [128, 2]     i32  {2*poi, 2*poi+1} for the core's cells
This makes everything core-local: the empty predicate is a [128, 512] u8
reduce-max over the cell's packed mask row (bit-packing on the host is
lossless layout marshalling), the poi gather is 2 x 128 8KB-half-row
descriptors via stock SWDGE indirect DMA (big rows amortize per-descriptor
cost; two instructions let the first half's compute start earlier; the
dma_gather ucode was rejected because its per-execution overlay load costs
~14us), and there is no collective — per-core runtime is independent of
cross-core launch skew (an AllReduce variant measured 66us of peer-wait).

Per-core HBM traffic: 2MB slice + 2MB gather + 64KB mask + 2MB out.
"""

import numpy as np

from concourse import bacc, bass, mybir, tile
from concourse.bass_utils import run_bass_kernel_spmd

N_CORES = 8
B, T, C, S2 = 64, 12, 64, 1024
SIDE = 32
ALL_ROWS = B * C                # 4096 (b, c) rows per cell
PACKED = ALL_ROWS // 8          # 512 packed mask bytes per cell
P = 128                         # SBUF partitions = cells per core
NCH = 4                         # row-chunks for the add/store pipeline
CHW = ALL_ROWS // NCH           # 1024 rows per chunk

_CACHE = {}


def _build_program():
    nc = bacc.Bacc(
        "TRN2",
        target_bir_lowering=False,
        debug=False,
        num_devices=N_CORES,
    )
    # data_full viewed as half-rows [2048, 2048]: cell c's columns
    # [0, 2048) live in row 2c, columns [2048, 4096) in row 2c+1.
    data_half = nc.dram_tensor(
        "data_half", [2 * S2, ALL_ROWS // 2], mybir.dt.float32, kind="ExternalInput"
    ).ap()
    data_slice = nc.dram_tensor(
        "data_slice", [P, ALL_ROWS], mybir.dt.float32, kind="ExternalInput"
    ).ap()
    maskp = nc.dram_tensor(
        "maskp", [P, PACKED], mybir.dt.uint8, kind="ExternalInput"
    ).ap()
    # idx2[p, 0] = 2*poi[cell], idx2[p, 1] = 2*poi[cell] + 1
    idx2 = nc.dram_tensor("idx2", [P, 2], mybir.dt.int32, kind="ExternalInput").ap()
    out_t = nc.dram_tensor(
        "out_t", [P, ALL_ROWS], mybir.dt.float32, kind="ExternalOutput"
    ).ap()

    with tile.TileContext(nc) as tc:
        with tc.tile_pool(name="sbuf", bufs=1) as pool:
            idx_sb = pool.tile([P, 2], mybir.dt.int32, tag="idx")
            nc.scalar.dma_start(out=idx_sb[:], in_=idx2[:])

            # gta[p, :] | gtb[p, :] = data_full[poi[128k + p], :2048 | 2048:]
            # Row gather as two stock SWDGE indirect DMAs of 8KB half-rows,
            # so downstream compute on the first half starts earlier.
            gts = []
            for h in range(2):
                gth = pool.tile([P, ALL_ROWS // 2], mybir.dt.float32, tag=f"g{h}")
                nc.gpsimd.indirect_dma_start(
                    out=gth[:],
                    out_offset=None,
                    in_=data_half[:, :],
                    in_offset=bass.IndirectOffsetOnAxis(
                        ap=idx_sb[:, h : h + 1], axis=0
                    ),
                )
                gts.append(gth)

            # ---- per-cell empty predicate (core-local) ----
            mp = pool.tile([P, PACKED], mybir.dt.uint8, tag="mask")
            nc.sync.dma_start(out=mp[:], in_=maskp[:])
            mmax = pool.tile([P, 1], mybir.dt.float32, tag="mmax")
            nc.vector.tensor_reduce(
                out=mmax[:],
                in_=mp[:],
                axis=mybir.AxisListType.X,
                op=mybir.AluOpType.max,
            )
            empty = pool.tile([P, 1], mybir.dt.float32, tag="empty")
            nc.vector.tensor_scalar(
                out=empty[:],
                in0=mmax[:],
                scalar1=0.0,
                scalar2=None,
                op0=mybir.AluOpType.is_equal,
            )

            # ---- data loads, chunked over rows ----
            dcs = []
            for c in range(NCH):
                dc = pool.tile([P, CHW], mybir.dt.float32, tag=f"d{c}")
                nc.sync.dma_start(
                    out=dc[:], in_=data_slice[:, c * CHW : (c + 1) * CHW]
                )
                dcs.append(dc)

            # ---- out = data + empty * gathered; mul on ACT, add on DVE ----
            half_chw = ALL_ROWS // 2 // (NCH // 2)
            for c in range(NCH):
                dc = dcs[c]
                gth = gts[c // (NCH // 2)]
                ci = c % (NCH // 2)
                gq = gth[:, ci * half_chw : (ci + 1) * half_chw]
                nc.scalar.mul(gq, gq, empty[:, 0:1])
                nc.vector.tensor_add(out=dc[:], in0=dc[:], in1=gq)
                nc.sync.dma_start(
                    out=out_t[:, c * CHW : (c + 1) * CHW], in_=dc[:]
                )

    nc.compile()
    return nc


def _get_program():
    if "nc" not in _CACHE:
        _CACHE["nc"] = _build_program()
    return _CACHE["nc"]


def _marshal(d, m, poi_index):
    d = np.asarray(d)
    m = np.asarray(m)
    poi_index = np.asarray(poi_index)

    # Full transposed views: [1024 cells, 4096 rows]
    data_full = np.ascontiguousarray(
        d[:, -1].reshape(ALL_ROWS, S2).T
    ).astype(np.float32)
    maskp_full = np.packbits(
        m[:, -1].reshape(ALL_ROWS, S2).T != 0, axis=1
    )  # [1024, 512] u8

    poi = poi_index.astype(np.int32)

    data_half = data_full.reshape(2 * S2, ALL_ROWS // 2)  # view, no copy

    in_maps = []
    for k in range(N_CORES):
        cells = slice(k * P, (k + 1) * P)
        idx2 = np.ascontiguousarray(
            np.stack([2 * poi[cells], 2 * poi[cells] + 1], axis=1)
        )  # [128, 2]
        in_maps.append(
            {
                "data_half": data_half,
                "data_slice": data_full[cells],
                "maskp": maskp_full[cells],
                "idx2": idx2,
            }
        )
    return in_maps


def _unmarshal(results):
    # results[k]["out_t"] is [128 cells, 4096 rows]; rows = b*64 + c.
    out = np.concatenate(
        [np.asarray(r["out_t"]) for r in results], axis=0
    )  # [1024, 4096]
    out = out.T.reshape(B, C, S2)  # [64, 64, 1024]
    return np.ascontiguousarray(out.reshape(B, C, SIDE, SIDE).astype(np.float32))


def run(d, m, poi_index, side, trace=False):
    """Run the Bass kernel; returns (output, BassKernelResults)."""
    nc = _get_program()
    in_maps = _marshal(d, m, poi_index)
    res = run_bass_kernel_spmd(
        nc, in_maps, list(range(N_CORES)), trace=trace
    )
    return _unmarshal(res.results), res


def kernel(d, m, poi_index, side):
    out, _ = run(d, m, poi_index, side)
    return out


# revision 28
# speedup vs baseline: 3.5406x; 1.1583x over previous
"""Trainium2 Bass kernel for nn_AuxCMP_61907658604772 (retrieval_knn).

Reference semantics (only the last time step of d/m matters):
    data = d[:, -1].reshape(B, C, S2)            # [64, 64, 1024] f32
    mask = m[:, -1].reshape(B, C, S2)            # [64, 64, 1024] i32 (0/1)
    cell_empty = (mask.sum(axis=(0, 1)) == 0)    # [1024] per-cell predicate
    gathered = data[:, :, poi_index]             # gather along cell dim
    out = (data + where(cell_empty, gathered, 0)).reshape(B, C, 32, 32)

Sharding: by CELLS — core k owns cells [128k, 128(k+1)) x all 4096 (b, c)
rows, in cell-major ("transposed") layout:
    data_half  [2048, 2048] f32  transposed d[:, -1], half-row view (replicated)
    data_slice [128, 4096]  f32  the core's own cell rows (shard)
    maskp      [128, 512]   u8   bit-packed mask rows for the core's cells
    idx2       [128, 2]     i32  {2*poi, 2*poi+1} for the core's cells
This makes everything core-local: the empty predicate is a [128, 512] u8
reduce-max over the cell's packed mask row (bit-packing on the host is
lossless layout marshalling), the poi gather is 2 x 128 8KB-half-row
descriptors via stock SWDGE indirect DMA (big rows amortize per-descriptor
cost; two instructions let the first half's compute start earlier; the
dma_gather ucode was rejected because its per-execution overlay load costs
~14us), and there is no collective — per-core runtime is independent of
cross-core launch skew (an AllReduce variant measured 66us of peer-wait).

Per-core HBM traffic: 2MB slice + 2MB gather + 64KB mask + 2MB out.
"""

import numpy as np

from concourse import bacc, bass, mybir, tile
from concourse.bass_utils import run_bass_kernel_spmd

N_CORES = 8
B, T, C, S2 = 64, 12, 64, 1024
SIDE = 32
ALL_ROWS = B * C                # 4096 (b, c) rows per cell
PACKED = ALL_ROWS // 8          # 512 packed mask bytes per cell
P = 128                         # SBUF partitions = cells per core
NCH = 4                         # row-chunks for the add/store pipeline
CHW = ALL_ROWS // NCH           # 1024 rows per chunk
NG = 4                          # gather split (quarter-rows)

_CACHE = {}


def _build_program():
    nc = bacc.Bacc(
        "TRN2",
        target_bir_lowering=False,
        debug=False,
        num_devices=N_CORES,
    )
    # data_full viewed as half-rows [2048, 2048]: cell c's columns
    # [2048*h, 2048*(h+1)) live in row 2c + h.
    data_q = nc.dram_tensor(
        "data_q", [NG * S2, ALL_ROWS // NG], mybir.dt.float32, kind="ExternalInput"
    ).ap()
    data_slice = nc.dram_tensor(
        "data_slice", [P, ALL_ROWS], mybir.dt.float32, kind="ExternalInput"
    ).ap()
    maskp = nc.dram_tensor(
        "maskp", [P, PACKED], mybir.dt.uint8, kind="ExternalInput"
    ).ap()
    # idx4[p, h] = NG*poi[cell] + h
    idx4 = nc.dram_tensor("idx4", [P, NG], mybir.dt.int32, kind="ExternalInput").ap()
    out_t = nc.dram_tensor(
        "out_t", [P, ALL_ROWS], mybir.dt.float32, kind="ExternalOutput"
    ).ap()

    with tile.TileContext(nc) as tc:
        with tc.tile_pool(name="sbuf", bufs=1) as pool:
            idx_sb = pool.tile([P, NG], mybir.dt.int32, tag="idx")
            nc.scalar.dma_start(out=idx_sb[:], in_=idx4[:])

            # ---- per-cell empty predicate (core-local) ----
            mp = pool.tile([P, PACKED], mybir.dt.uint8, tag="mask")
            nc.sync.dma_start(out=mp[:], in_=maskp[:])
            mmax = pool.tile([P, 1], mybir.dt.float32, tag="mmax")
            nc.vector.tensor_reduce(
                out=mmax[:],
                in_=mp[:],
                axis=mybir.AxisListType.X,
                op=mybir.AluOpType.max,
            )
            empty = pool.tile([P, 1], mybir.dt.float32, tag="empty")
            nc.vector.tensor_scalar(
                out=empty[:],
                in0=mmax[:],
                scalar1=0.0,
                scalar2=None,
                op0=mybir.AluOpType.is_equal,
            )

            # idx_eff = idx4 + (1 - empty) * 65536: non-empty cells' indices
            # pushed out of bounds so their gather descriptors are skipped
            # (bounds_check + oob_is_err=False) — halves gather traffic.
            shift = pool.tile([P, 1], mybir.dt.float32, tag="shift")
            nc.vector.tensor_scalar(
                out=shift[:],
                in0=empty[:],
                scalar1=-65536.0,
                scalar2=65536.0,
                op0=mybir.AluOpType.mult,
                op1=mybir.AluOpType.add,
            )
            idx_f = pool.tile([P, NG], mybir.dt.float32, tag="idxf")
            nc.vector.tensor_copy(out=idx_f[:], in_=idx_sb[:])
            nc.vector.tensor_scalar(
                out=idx_f[:],
                in0=idx_f[:],
                scalar1=shift[:, 0:1],
                scalar2=None,
                op0=mybir.AluOpType.add,
            )
            idx_eff = pool.tile([P, NG], mybir.dt.int32, tag="idxe")
            nc.vector.tensor_copy(out=idx_eff[:], in_=idx_f[:])

            # gts[q][p, :] = data_full[poi[128k + p], 1024q : 1024(q+1)]
            # for empty cells; stays zero (memset) for skipped ones.
            # Four stock SWDGE indirect DMAs of 4KB quarter-rows, so compute
            # on each column chunk starts as soon as its gather lands.
            gts = []
            for h in range(NG):
                gth = pool.tile([P, ALL_ROWS // NG], mybir.dt.float32, tag=f"g{h}")
                nc.scalar.memzero(gth[:])
                nc.gpsimd.indirect_dma_start(
                    out=gth[:],
                    out_offset=None,
                    in_=data_q[:, :],
                    in_offset=bass.IndirectOffsetOnAxis(
                        ap=idx_eff[:, h : h + 1], axis=0
                    ),
                    bounds_check=NG * S2 - 1,
                    oob_is_err=False,
                )
                gts.append(gth)

            # ---- data loads, chunked over rows ----
            dcs = []
            for c in range(NCH):
                dc = pool.tile([P, CHW], mybir.dt.float32, tag=f"d{c}")
                nc.sync.dma_start(
                    out=dc[:], in_=data_slice[:, c * CHW : (c + 1) * CHW]
                )
                dcs.append(dc)

            # ---- out = data + empty * gathered, fused on DVE ----
            per_g = NCH // NG
            for c in range(NCH):
                dc = dcs[c]
                gq = gts[c // per_g][:, (c % per_g) * CHW : (c % per_g + 1) * CHW]
                nc.vector.scalar_tensor_tensor(
                    out=dc[:],
                    in0=gq,
                    scalar=empty[:, 0:1],
                    in1=dc[:],
                    op0=mybir.AluOpType.mult,
                    op1=mybir.AluOpType.add,
                )
                nc.sync.dma_start(
                    out=out_t[:, c * CHW : (c + 1) * CHW], in_=dc[:]
                )

    nc.compile()
    return nc


def _get_program():
    if "nc" not in _CACHE:
        _CACHE["nc"] = _build_program()
    return _CACHE["nc"]


def _marshal(d, m, poi_index):
    d = np.asarray(d)
    m = np.asarray(m)
    poi_index = np.asarray(poi_index)

    # Full transposed views: [1024 cells, 4096 rows]
    data_full = np.ascontiguousarray(
        d[:, -1].reshape(ALL_ROWS, S2).T
    ).astype(np.float32)
    maskp_full = np.packbits(
        m[:, -1].reshape(ALL_ROWS, S2).T != 0, axis=1
    )  # [1024, 512] u8

    poi = poi_index.astype(np.int32)

    data_q = data_full.reshape(NG * S2, ALL_ROWS // NG)  # view, no copy

    in_maps = []
    for k in range(N_CORES):
        cells = slice(k * P, (k + 1) * P)
        idx4 = np.ascontiguousarray(
            NG * poi[cells, None] + np.arange(NG, dtype=np.int32)[None, :]
        )  # [128, NG]
        in_maps.append(
            {
                "data_q": data_q,
                "data_slice": data_full[cells],
                "maskp": maskp_full[cells],
                "idx4": idx4,
            }
        )
    return in_maps


def _unmarshal(results):
    # results[k]["out_t"] is [128 cells, 4096 rows]; rows = b*64 + c.
    out = np.concatenate(
        [np.asarray(r["out_t"]) for r in results], axis=0
    )  # [1024, 4096]
    out = out.T.reshape(B, C, S2)  # [64, 64, 1024]
    return np.ascontiguousarray(out.reshape(B, C, SIDE, SIDE).astype(np.float32))


def run(d, m, poi_index, side, trace=False):
    """Run the Bass kernel; returns (output, BassKernelResults)."""
    nc = _get_program()
    in_maps = _marshal(d, m, poi_index)
    res = run_bass_kernel_spmd(
        nc, in_maps, list(range(N_CORES)), trace=trace
    )
    return _unmarshal(res.results), res


def kernel(d, m, poi_index, side):
    out, _ = run(d, m, poi_index, side)
    return out
